# revision 31
# baseline (speedup 1.0000x reference)
"""Trainium2 kernel for nn_CascadedABCDCircuit: cascaded 2-port ABCD ladder.

Math: each stage multiplies the ABCD state by (I + s_i*G_i) where G_i is a
constant nilpotent 2x2 complex matrix and s_i = (omega*v_i)^{+-1}. Every
output component is therefore a Laurent polynomial in omega, degree -6..+6,
with batch-dependent coefficients. Host computes the 13 coefficients per
(component, batch) exactly in fp64 via the recurrence applied to polynomial
coefficient vectors (tiny (1024,13) complex ops). The device evaluates
out[c,b,f] = sum_m C[c,b,m] * W[m,f] as K=13 fp32r matmuls and streams the
256MB result to HBM — memory-bound, as this problem's regime demands.

Precision: plain fp32r (no hi/lo operand splits). fp32r rounds operands to
~12 mantissa bits; the measured end-to-end rel_l2 is ~1e-4, far inside the
2e-2 gate, and the single-term form cuts the input stream to 0.46MB/core.

Sharding: pure data-parallel over batch: 8 cores x 128 batches, every core
sees all 8192 freqs. The pipeline is tuned for the ~420 GB/s per-core DMA
write wall: production runs in 1024-col PSUM chunks (4-deep rotation)
copied by DVE/ACT alternately into 4096-col staging tiles (6 deep), each
shipped as ONE output DMA — 19 output DMAs total keeps the 8-deep HWDGE
completion-semaphore reuse window from ever stalling the two hardware
issue rings (sync/scalar). Groups go out f-block-major (every component's
[0:4096] block before any [4096:] block) so the first half of production
depends only on the first two wst loads — DGE completion semaphores have
a multi-us latency and mid-ramp waits on them starve the drain. The DRAM
pad keeps the output base in the HBM address phase where all 16 DMA
engines run at full per-engine rate (without it one engine runs ~20%
slow and becomes the long pole; the phase also depends on runtime
allocator state, so this is best-effort).
"""
import numpy as np
import sys

for _p in ("/opt/trn_rl_repo", "/root/.axon_site/_ro/trn_rl_repo"):
    if _p not in sys.path:
        sys.path.append(_p)

import concourse.bacc as bacc
import concourse.mybir as mybir
from concourse import tile
from concourse.bass_utils import run_bass_kernel_spmd

# Problem constants (hardcoded per contract)
B, F = 1024, 8192
OP_CODES = [3, 0, 1, 2, 3, 0, 1, 2, 3, 0, 1, 2]
Q_L, Q_C = 50.0, 100.0
NK, K0 = 13, 6               # omega powers -6..+6
NCORES = 8
BPC = B // NCORES            # 128 batches per core
NCOMP = 8                    # Ar, Ai, Br, Bi, Cr, Ci, Dr, Di
OM0 = 2.0 * np.pi * np.sqrt(1e9 * 10e9)   # omega normalizer (geometric mid)

MM_N = 512                   # max matmul moving cols (1 PSUM bank, fp32)
CHUNK = 1024                 # PSUM chunk cols (2 PSUM banks)
GRP = 4096                   # staging/DMA group cols
CB = NCOMP * BPC             # coefficient columns (1024)

LAST_RESULTS = None          # BassKernelResults of the most recent run
_COMPILED = {}


def _host_coeffs(values):
    """values (B,12) fp32 -> (NCOMP, B, NK) fp64 coeffs in powers of (om/OM0)."""
    v = values.astype(np.float64)
    nb = v.shape[0]
    A = np.zeros((nb, NK), np.complex128); A[:, K0] = 1.0
    Bm = np.zeros((nb, NK), np.complex128)
    Cm = np.zeros((nb, NK), np.complex128)
    Dm = np.zeros((nb, NK), np.complex128); Dm[:, K0] = 1.0

    def shift_mul(P, fac, dk):
        out = np.zeros_like(P)
        if dk == 1:
            out[:, 1:] = P[:, :-1]
        else:
            out[:, :-1] = P[:, 1:]
        return out * fac[:, None]

    for i, code in enumerate(OP_CODES):
        vi = v[:, i]
        if code == 0:      # series L
            fac = vi * OM0 * (1.0 / Q_L + 1j)
            Bm = Bm + shift_mul(A, fac, +1)
            Dm = Dm + shift_mul(Cm, fac, +1)
        elif code == 1:    # series C (reciprocal)
            c = (1.0 / Q_C - 1j) / (1.0 + 1.0 / Q_C**2)
            fac = c / (vi * OM0)
            Bm = Bm + shift_mul(A, fac, -1)
            Dm = Dm + shift_mul(Cm, fac, -1)
        elif code == 2:    # shunt L (reciprocal)
            c = (1.0 / Q_L - 1j) / (1.0 + 1.0 / Q_L**2)
            fac = c / (vi * OM0)
            A = A + shift_mul(Bm, fac, -1)
            Cm = Cm + shift_mul(Dm, fac, -1)
        else:              # shunt C
            fac = vi * OM0 * (1.0 / Q_C + 1j)
            A = A + shift_mul(Bm, fac, +1)
            Cm = Cm + shift_mul(Dm, fac, +1)
    return np.stack([A.real, A.imag, Bm.real, Bm.imag,
                     Cm.real, Cm.imag, Dm.real, Dm.imag])


def _group_schedule():
    """(c, pos, width) DMA groups, f-block-major: every component's
    [0:4096] block first, then the [4096:8192] blocks. The first half of
    production then depends only on the first two wst loads — no mid-ramp
    waits on load-completion semaphores. c0 leads with a small group so
    output DMA starts early; the last component tapers so the final queue
    drain is short."""
    groups = [(0, 0, 1024), (0, 1024, 3072)]
    for c in range(1, NCOMP):
        groups.append((c, 0, 4096))
    for c in range(NCOMP - 1):
        groups.append((c, 4096, 4096))
    groups += [(NCOMP - 1, 4096, 2048), (NCOMP - 1, 6144, 1024),
               (NCOMP - 1, 7168, 1024)]
    return groups


def _build_module():
    """SPMD module: cst[13,CB] + wst[13,F] -> out[NCOMP, BPC, F]."""
    nc = bacc.Bacc("TRN2", target_bir_lowering=False, debug=False,
                   enable_asserts=False, num_devices=NCORES)
    cst_d = nc.dram_tensor("cst", [NK, CB], mybir.dt.float32r,
                           kind="ExternalInput")
    wst_d = nc.dram_tensor("wst", [NK, F], mybir.dt.float32r,
                           kind="ExternalInput")
    # pad the DRAM layout so `out` starts at the same HBM offset class as
    # the fast baseline (inputs totalled 1.4375MB there); without this one
    # DMA engine's address slice lands ~24% slower and becomes the long pole
    pad_bytes = int(1.4375 * 1024 * 1024) - NK * CB * 4 - NK * F * 4
    nc.dram_tensor("pad", [pad_bytes // 4], mybir.dt.float32,
                   kind="Internal")
    out_d = nc.dram_tensor("out", [NCOMP, BPC, F], mybir.dt.float32,
                           kind="ExternalOutput")

    with tile.TileContext(nc) as tc:
        with (
            tc.tile_pool(name="const", bufs=1) as cpool,
            tc.tile_pool(name="stage", bufs=5) as spool,
            tc.tile_pool(name="ps", bufs=4, space="PSUM") as pspool,
        ):
            cst = cpool.tile([NK, CB], mybir.dt.float32r)
            wst = cpool.tile([NK, F], mybir.dt.float32r)
            nc.scalar.dma_start(wst[:, :1024], wst_d[:, :1024])
            nc.sync.dma_start(cst[:, :BPC], cst_d[:, :BPC])
            nc.scalar.dma_start(wst[:, 1024:4096], wst_d[:, 1024:4096])
            nc.sync.dma_start(cst[:, BPC:], cst_d[:, BPC:])
            nc.scalar.dma_start(wst[:, 4096:F], wst_d[:, 4096:F])

            # Production runs in 1024-col PSUM chunks (4-deep rotation);
            # DMA is issued per 4096-col staging group: only 19 output DMAs,
            # so the 8-deep HWDGE completion-semaphore window (sem-reuse
            # guards on the in-order issue rings) spans ~16MB of drain.
            for ng, (c, gpos, gw) in enumerate(_group_schedule()):
                ot = spool.tile([BPC, GRP], mybir.dt.float32)
                lhsT = cst[:, c * BPC:(c + 1) * BPC]
                nq = (gw + CHUNK - 1) // CHUNK
                for qi in range(nq):
                    q = qi * CHUNK
                    w = min(CHUNK, gw - q)
                    pos = gpos + q
                    acc = pspool.tile([BPC, CHUNK], mybir.dt.float32)
                    for j in range(0, w, MM_N):
                        mw = min(MM_N, w - j)
                        nc.tensor.matmul(acc[:, j:j + mw],
                                         lhsT, wst[:, pos + j:pos + j + mw])
                    # PSUM is only readable by DVE/ACT: alternate for copies
                    if (ng * 4 + qi) % 2 == 0:
                        nc.vector.tensor_copy(ot[:, q:q + w], acc[:, :w])
                    else:
                        nc.scalar.copy(ot[:, q:q + w], acc[:, :w])
                # only sync and scalar own hardware DGE rings
                eng = nc.sync if ng % 2 == 0 else nc.scalar
                eng.dma_start(out_d[c, :, gpos:gpos + gw], ot[:, :gw])
    nc.compile()
    return nc


def kernel(values: np.ndarray, freq_hz: np.ndarray) -> np.ndarray:
    global LAST_RESULTS
    values = np.asarray(values, np.float32)
    freq_hz = np.asarray(freq_hz, np.float32)
    assert values.shape == (B, len(OP_CODES)) and freq_hz.shape == (F,)

    # Host precompute (tiny, fp64-exact): Laurent coefficients + omega powers
    coef = _host_coeffs(values)                              # (8, B, 13) f64
    om = 2.0 * np.pi * freq_hz.astype(np.float64)
    wt = om / OM0
    W = np.stack([wt ** (k - K0) for k in range(NK)]).astype(np.float32)

    if "nc" not in _COMPILED:
        _COMPILED["nc"] = _build_module()
    nc = _COMPILED["nc"]

    in_maps = []
    for core in range(NCORES):
        sl = slice(core * BPC, (core + 1) * BPC)
        cstack = np.ascontiguousarray(
            np.transpose(coef[:, sl, :], (2, 0, 1)).reshape(NK, CB)
        ).astype(np.float32)
        in_maps.append({"cst": cstack, "wst": W})

    res = run_bass_kernel_spmd(nc, in_maps, core_ids=list(range(NCORES)))
    LAST_RESULTS = res
    out = np.concatenate([res.results[c]["out"] for c in range(NCORES)], axis=1)
    return out.astype(np.float32, copy=False)


# revision 32
# speedup vs baseline: 1.0674x; 1.0674x over previous
"""Trainium2 kernel for nn_CascadedABCDCircuit: cascaded 2-port ABCD ladder.

Math: each stage multiplies the ABCD state by (I + s_i*G_i) where G_i is a
constant nilpotent 2x2 complex matrix and s_i = (omega*v_i)^{+-1}. Every
output component is therefore a Laurent polynomial in omega, degree -6..+6,
with batch-dependent coefficients. Host computes the 13 coefficients per
(component, batch) exactly in fp64 via the recurrence applied to polynomial
coefficient vectors (tiny (1024,13) complex ops). The device evaluates
out[c,b,f] = sum_m C[c,b,m] * W[m,f] as K=13 fp32r matmuls and streams the
256MB result to HBM — memory-bound, as this problem's regime demands.

Precision: plain fp32r (no hi/lo operand splits). fp32r rounds operands to
~12 mantissa bits; the measured end-to-end rel_l2 is ~1e-4, far inside the
2e-2 gate, and the single-term form cuts the input stream to 0.46MB/core.

Sharding: pure data-parallel over batch: 8 cores x 128 batches, every core
sees all 8192 freqs. The pipeline is tuned for the ~420 GB/s per-core DMA
write wall: production runs in 1024-col PSUM chunks (4-deep rotation)
copied by DVE/ACT alternately into 4096-col staging tiles (6 deep), each
shipped as ONE output DMA — 19 output DMAs total keeps the 8-deep HWDGE
completion-semaphore reuse window from ever stalling the two hardware
issue rings (sync/scalar). Groups go out f-block-major (every component's
[0:4096] block before any [4096:] block) so the first half of production
depends only on the first two wst loads — DGE completion semaphores have
a multi-us latency and mid-ramp waits on them starve the drain. The DRAM
pad keeps the output base in the HBM address phase where all 16 DMA
engines run at full per-engine rate (without it one engine runs ~20%
slow and becomes the long pole; the phase also depends on runtime
allocator state, so this is best-effort).
"""
import numpy as np
import sys

for _p in ("/opt/trn_rl_repo", "/root/.axon_site/_ro/trn_rl_repo"):
    if _p not in sys.path:
        sys.path.append(_p)

import concourse.bacc as bacc
import concourse.mybir as mybir
from concourse import tile
from concourse.bass_utils import run_bass_kernel_spmd

# Problem constants (hardcoded per contract)
B, F = 1024, 8192
OP_CODES = [3, 0, 1, 2, 3, 0, 1, 2, 3, 0, 1, 2]
Q_L, Q_C = 50.0, 100.0
NK, K0 = 13, 6               # omega powers -6..+6
NCORES = 8
BPC = B // NCORES            # 128 batches per core
NCOMP = 8                    # Ar, Ai, Br, Bi, Cr, Ci, Dr, Di
OM0 = 2.0 * np.pi * np.sqrt(1e9 * 10e9)   # omega normalizer (geometric mid)

MM_N = 512                   # max matmul moving cols (1 PSUM bank, fp32)
CHUNK = 1024                 # PSUM chunk cols (2 PSUM banks)
GRP = 4096                   # staging/DMA group cols
CB = NCOMP * BPC             # coefficient columns (1024)

LAST_RESULTS = None          # BassKernelResults of the most recent run
_COMPILED = {}


def _host_coeffs(values):
    """values (B,12) fp32 -> (NCOMP, B, NK) fp64 coeffs in powers of (om/OM0)."""
    v = values.astype(np.float64)
    nb = v.shape[0]
    A = np.zeros((nb, NK), np.complex128); A[:, K0] = 1.0
    Bm = np.zeros((nb, NK), np.complex128)
    Cm = np.zeros((nb, NK), np.complex128)
    Dm = np.zeros((nb, NK), np.complex128); Dm[:, K0] = 1.0

    def shift_mul(P, fac, dk):
        out = np.zeros_like(P)
        if dk == 1:
            out[:, 1:] = P[:, :-1]
        else:
            out[:, :-1] = P[:, 1:]
        return out * fac[:, None]

    for i, code in enumerate(OP_CODES):
        vi = v[:, i]
        if code == 0:      # series L
            fac = vi * OM0 * (1.0 / Q_L + 1j)
            Bm = Bm + shift_mul(A, fac, +1)
            Dm = Dm + shift_mul(Cm, fac, +1)
        elif code == 1:    # series C (reciprocal)
            c = (1.0 / Q_C - 1j) / (1.0 + 1.0 / Q_C**2)
            fac = c / (vi * OM0)
            Bm = Bm + shift_mul(A, fac, -1)
            Dm = Dm + shift_mul(Cm, fac, -1)
        elif code == 2:    # shunt L (reciprocal)
            c = (1.0 / Q_L - 1j) / (1.0 + 1.0 / Q_L**2)
            fac = c / (vi * OM0)
            A = A + shift_mul(Bm, fac, -1)
            Cm = Cm + shift_mul(Dm, fac, -1)
        else:              # shunt C
            fac = vi * OM0 * (1.0 / Q_C + 1j)
            A = A + shift_mul(Bm, fac, +1)
            Cm = Cm + shift_mul(Dm, fac, +1)
    return np.stack([A.real, A.imag, Bm.real, Bm.imag,
                     Cm.real, Cm.imag, Dm.real, Dm.imag])


def _group_schedule():
    """(c, pos, width) DMA groups, f-block-major: every component's
    [0:4096] block first, then the [4096:8192] blocks. The first half of
    production then depends only on the first two wst loads — no mid-ramp
    waits on load-completion semaphores. c0 leads with a small group so
    output DMA starts early; the last component tapers so the final queue
    drain is short."""
    groups = [(0, 0, 1024), (0, 1024, 3072)]
    for c in range(1, NCOMP):
        groups.append((c, 0, 4096))
    for c in range(NCOMP - 1):
        groups.append((c, 4096, 4096))
    groups += [(NCOMP - 1, 4096, 2048), (NCOMP - 1, 6144, 1024),
               (NCOMP - 1, 7168, 1024)]
    return groups


def _build_module():
    """SPMD module: cst[13,CB] + wst[13,F] -> out[NCOMP, BPC, F]."""
    nc = bacc.Bacc("TRN2", target_bir_lowering=False, debug=False,
                   enable_asserts=False, num_devices=NCORES)
    cst_d = nc.dram_tensor("cst", [NK, CB], mybir.dt.float32r,
                           kind="ExternalInput")
    wst_d = nc.dram_tensor("wst", [NK, F], mybir.dt.float32r,
                           kind="ExternalInput")
    # pad the DRAM layout so `out` starts at the same HBM offset class as
    # the fast baseline (inputs totalled 1.4375MB there); without this one
    # DMA engine's address slice lands ~24% slower and becomes the long pole
    pad_bytes = int(1.4375 * 1024 * 1024) - NK * CB * 4 - NK * F * 4
    nc.dram_tensor("pad", [pad_bytes // 4], mybir.dt.float32,
                   kind="Internal")
    out_d = nc.dram_tensor("out", [NCOMP, BPC, F], mybir.dt.float32,
                           kind="ExternalOutput")

    with tile.TileContext(nc) as tc:
        with (
            tc.tile_pool(name="const", bufs=1) as cpool,
            tc.tile_pool(name="stage", bufs=6) as spool,
            tc.tile_pool(name="ps", bufs=4, space="PSUM") as pspool,
        ):
            cst = cpool.tile([NK, CB], mybir.dt.float32r)
            wst = cpool.tile([NK, F], mybir.dt.float32r)
            nc.scalar.dma_start(wst[:, :1024], wst_d[:, :1024])
            nc.sync.dma_start(cst[:, :BPC], cst_d[:, :BPC])
            nc.scalar.dma_start(wst[:, 1024:4096], wst_d[:, 1024:4096])
            nc.sync.dma_start(cst[:, BPC:], cst_d[:, BPC:])
            nc.scalar.dma_start(wst[:, 4096:F], wst_d[:, 4096:F])

            # Production runs in 1024-col PSUM chunks (4-deep rotation);
            # DMA is issued per 4096-col staging group: only 19 output DMAs,
            # so the 8-deep HWDGE completion-semaphore window (sem-reuse
            # guards on the in-order issue rings) spans ~16MB of drain.
            for ng, (c, gpos, gw) in enumerate(_group_schedule()):
                ot = spool.tile([BPC, GRP], mybir.dt.float32)
                lhsT = cst[:, c * BPC:(c + 1) * BPC]
                nq = (gw + CHUNK - 1) // CHUNK
                for qi in range(nq):
                    q = qi * CHUNK
                    w = min(CHUNK, gw - q)
                    pos = gpos + q
                    acc = pspool.tile([BPC, CHUNK], mybir.dt.float32)
                    for j in range(0, w, MM_N):
                        mw = min(MM_N, w - j)
                        nc.tensor.matmul(acc[:, j:j + mw],
                                         lhsT, wst[:, pos + j:pos + j + mw])
                    # PSUM is only readable by DVE/ACT: alternate for copies
                    if (ng * 4 + qi) % 2 == 0:
                        nc.vector.tensor_copy(ot[:, q:q + w], acc[:, :w])
                    else:
                        nc.scalar.copy(ot[:, q:q + w], acc[:, :w])
                # only sync and scalar own hardware DGE rings
                eng = nc.sync if ng % 2 == 0 else nc.scalar
                eng.dma_start(out_d[c, :, gpos:gpos + gw], ot[:, :gw])
    nc.compile()
    return nc


def kernel(values: np.ndarray, freq_hz: np.ndarray) -> np.ndarray:
    global LAST_RESULTS
    values = np.asarray(values, np.float32)
    freq_hz = np.asarray(freq_hz, np.float32)
    assert values.shape == (B, len(OP_CODES)) and freq_hz.shape == (F,)

    # Host precompute (tiny, fp64-exact): Laurent coefficients + omega powers
    coef = _host_coeffs(values)                              # (8, B, 13) f64
    om = 2.0 * np.pi * freq_hz.astype(np.float64)
    wt = om / OM0
    W = np.stack([wt ** (k - K0) for k in range(NK)]).astype(np.float32)

    if "nc" not in _COMPILED:
        _COMPILED["nc"] = _build_module()
    nc = _COMPILED["nc"]

    in_maps = []
    for core in range(NCORES):
        sl = slice(core * BPC, (core + 1) * BPC)
        cstack = np.ascontiguousarray(
            np.transpose(coef[:, sl, :], (2, 0, 1)).reshape(NK, CB)
        ).astype(np.float32)
        in_maps.append({"cst": cstack, "wst": W})

    res = run_bass_kernel_spmd(nc, in_maps, core_ids=list(range(NCORES)))
    LAST_RESULTS = res
    out = np.concatenate([res.results[c]["out"] for c in range(NCORES)], axis=1)
    return out.astype(np.float32, copy=False)


# revision 33
# speedup vs baseline: 1.1110x; 1.0409x over previous
"""Trainium2 kernel for nn_CascadedABCDCircuit: cascaded 2-port ABCD ladder.

Math: each stage multiplies the ABCD state by (I + s_i*G_i) where G_i is a
constant nilpotent 2x2 complex matrix and s_i = (omega*v_i)^{+-1}. Every
output component is therefore a Laurent polynomial in omega, degree -6..+6,
with batch-dependent coefficients. Host computes the 13 coefficients per
(component, batch) exactly in fp64 via the recurrence applied to polynomial
coefficient vectors (tiny (1024,13) complex ops). The device evaluates
out[c,b,f] = sum_m C[c,b,m] * W[m,f] as K=13 fp32r matmuls and streams the
256MB result to HBM — memory-bound, as this problem's regime demands.

Precision: plain fp32r (no hi/lo operand splits). fp32r rounds operands to
~12 mantissa bits; the measured end-to-end rel_l2 is ~1e-4, far inside the
2e-2 gate, and the single-term form cuts the input stream to 0.46MB/core.

Sharding: pure data-parallel over batch: 8 cores x 128 batches, every core
sees all 8192 freqs. The pipeline is tuned for the ~420 GB/s per-core DMA
write wall: production runs in 1024-col PSUM chunks (4-deep rotation)
copied by DVE/ACT alternately into 4096-col staging tiles (6 deep), each
shipped as ONE output DMA — 19 output DMAs total keeps the 8-deep HWDGE
completion-semaphore reuse window from ever stalling the two hardware
issue rings (sync/scalar). Groups go out f-block-major (every component's
[0:4096] block before any [4096:] block) so the first half of production
depends only on the first two wst loads — DGE completion semaphores have
a multi-us latency and mid-ramp waits on them starve the drain. The DRAM
pad keeps the output base in the HBM address phase where all 16 DMA
engines run at full per-engine rate (without it one engine runs ~20%
slow and becomes the long pole; the phase also depends on runtime
allocator state, so this is best-effort).
"""
import numpy as np
import sys

for _p in ("/opt/trn_rl_repo", "/root/.axon_site/_ro/trn_rl_repo"):
    if _p not in sys.path:
        sys.path.append(_p)

import concourse.bacc as bacc
import concourse.mybir as mybir
from concourse import tile
from concourse.bass_utils import run_bass_kernel_spmd

# Problem constants (hardcoded per contract)
B, F = 1024, 8192
OP_CODES = [3, 0, 1, 2, 3, 0, 1, 2, 3, 0, 1, 2]
Q_L, Q_C = 50.0, 100.0
NK, K0 = 13, 6               # omega powers -6..+6
NCORES = 8
BPC = B // NCORES            # 128 batches per core
NCOMP = 8                    # Ar, Ai, Br, Bi, Cr, Ci, Dr, Di
OM0 = 2.0 * np.pi * np.sqrt(1e9 * 10e9)   # omega normalizer (geometric mid)

MM_N = 512                   # max matmul moving cols (1 PSUM bank, fp32)
CHUNK = 1024                 # PSUM chunk cols (2 PSUM banks)
GRP = 4096                   # staging/DMA group cols
CB = NCOMP * BPC             # coefficient columns (1024)

LAST_RESULTS = None          # BassKernelResults of the most recent run
_COMPILED = {}


def _host_coeffs(values):
    """values (B,12) fp32 -> (NCOMP, B, NK) fp64 coeffs in powers of (om/OM0)."""
    v = values.astype(np.float64)
    nb = v.shape[0]
    A = np.zeros((nb, NK), np.complex128); A[:, K0] = 1.0
    Bm = np.zeros((nb, NK), np.complex128)
    Cm = np.zeros((nb, NK), np.complex128)
    Dm = np.zeros((nb, NK), np.complex128); Dm[:, K0] = 1.0

    def shift_mul(P, fac, dk):
        out = np.zeros_like(P)
        if dk == 1:
            out[:, 1:] = P[:, :-1]
        else:
            out[:, :-1] = P[:, 1:]
        return out * fac[:, None]

    for i, code in enumerate(OP_CODES):
        vi = v[:, i]
        if code == 0:      # series L
            fac = vi * OM0 * (1.0 / Q_L + 1j)
            Bm = Bm + shift_mul(A, fac, +1)
            Dm = Dm + shift_mul(Cm, fac, +1)
        elif code == 1:    # series C (reciprocal)
            c = (1.0 / Q_C - 1j) / (1.0 + 1.0 / Q_C**2)
            fac = c / (vi * OM0)
            Bm = Bm + shift_mul(A, fac, -1)
            Dm = Dm + shift_mul(Cm, fac, -1)
        elif code == 2:    # shunt L (reciprocal)
            c = (1.0 / Q_L - 1j) / (1.0 + 1.0 / Q_L**2)
            fac = c / (vi * OM0)
            A = A + shift_mul(Bm, fac, -1)
            Cm = Cm + shift_mul(Dm, fac, -1)
        else:              # shunt C
            fac = vi * OM0 * (1.0 / Q_C + 1j)
            A = A + shift_mul(Bm, fac, +1)
            Cm = Cm + shift_mul(Dm, fac, +1)
    return np.stack([A.real, A.imag, Bm.real, Bm.imag,
                     Cm.real, Cm.imag, Dm.real, Dm.imag])


def _group_schedule():
    """(c, pos, width) DMA groups, f-block-major: every component's
    [0:4096] block first, then the [4096:8192] blocks. The first half of
    production then depends only on the first two wst loads — no mid-ramp
    waits on load-completion semaphores. c0 leads with a small group so
    output DMA starts early; the last component tapers so the final queue
    drain is short."""
    groups = [(0, 0, 1024), (0, 1024, 3072)]
    for c in range(1, NCOMP):
        groups.append((c, 0, 4096))
    for c in range(NCOMP - 1):
        groups.append((c, 4096, 4096))
    groups += [(NCOMP - 1, 4096, 2048), (NCOMP - 1, 6144, 1024),
               (NCOMP - 1, 7168, 1024)]
    return groups


def _build_module():
    """SPMD module: cst[13,CB] + wst[13,F] -> out[NCOMP, BPC, F]."""
    nc = bacc.Bacc("TRN2", target_bir_lowering=False, debug=False,
                   enable_asserts=False, num_devices=NCORES)
    cst_d = nc.dram_tensor("cst", [NK, CB], mybir.dt.float32r,
                           kind="ExternalInput")
    wst_d = nc.dram_tensor("wst", [NK, F], mybir.dt.float32r,
                           kind="ExternalInput")
    # pad the DRAM layout so `out` starts at the same HBM offset class as
    # the fast baseline (inputs totalled 1.4375MB there); without this one
    # DMA engine's address slice lands ~24% slower and becomes the long pole
    pad_bytes = int(1.4375 * 1024 * 1024) - NK * CB * 4 - NK * F * 4
    nc.dram_tensor("pad", [pad_bytes // 4], mybir.dt.float32,
                   kind="Internal")
    out_d = nc.dram_tensor("out", [NCOMP, BPC, F], mybir.dt.float32,
                           kind="ExternalOutput")

    with tile.TileContext(nc) as tc:
        with (
            tc.tile_pool(name="const", bufs=1) as cpool,
            tc.tile_pool(name="stage", bufs=6) as spool,
            tc.tile_pool(name="ps", bufs=4, space="PSUM") as pspool,
        ):
            cst = cpool.tile([NK, CB], mybir.dt.float32r)
            wst = cpool.tile([NK, F], mybir.dt.float32r)
            # split load elements small (max_dma_last_dim) so every DGE
            # engine gets descriptors: a 13-row load otherwise leaves 3 of
            # 16 engines idle and its completion semaphore (all 16 must
            # report) lags the data by several us, starving the ramp
            nc.scalar.dma_start(wst[:, :1024], wst_d[:, :1024],
                                max_dma_last_dim=256)
            nc.sync.dma_start(cst[:, :BPC], cst_d[:, :BPC],
                              max_dma_last_dim=64)
            nc.scalar.dma_start(wst[:, 1024:4096], wst_d[:, 1024:4096],
                                max_dma_last_dim=512)
            nc.sync.dma_start(cst[:, BPC:], cst_d[:, BPC:],
                              max_dma_last_dim=256)
            nc.scalar.dma_start(wst[:, 4096:F], wst_d[:, 4096:F],
                                max_dma_last_dim=512)

            # Production runs in 1024-col PSUM chunks (4-deep rotation);
            # DMA is issued per 4096-col staging group: only 19 output DMAs,
            # so the 8-deep HWDGE completion-semaphore window (sem-reuse
            # guards on the in-order issue rings) spans ~16MB of drain.
            for ng, (c, gpos, gw) in enumerate(_group_schedule()):
                ot = spool.tile([BPC, GRP], mybir.dt.float32)
                lhsT = cst[:, c * BPC:(c + 1) * BPC]
                nq = (gw + CHUNK - 1) // CHUNK
                for qi in range(nq):
                    q = qi * CHUNK
                    w = min(CHUNK, gw - q)
                    pos = gpos + q
                    acc = pspool.tile([BPC, CHUNK], mybir.dt.float32)
                    for j in range(0, w, MM_N):
                        mw = min(MM_N, w - j)
                        nc.tensor.matmul(acc[:, j:j + mw],
                                         lhsT, wst[:, pos + j:pos + j + mw])
                    # PSUM is only readable by DVE/ACT: alternate for copies
                    if (ng * 4 + qi) % 2 == 0:
                        nc.vector.tensor_copy(ot[:, q:q + w], acc[:, :w])
                    else:
                        nc.scalar.copy(ot[:, q:q + w], acc[:, :w])
                # only sync and scalar own hardware DGE rings
                eng = nc.sync if ng % 2 == 0 else nc.scalar
                eng.dma_start(out_d[c, :, gpos:gpos + gw], ot[:, :gw])
    nc.compile()
    return nc


def kernel(values: np.ndarray, freq_hz: np.ndarray) -> np.ndarray:
    global LAST_RESULTS
    values = np.asarray(values, np.float32)
    freq_hz = np.asarray(freq_hz, np.float32)
    assert values.shape == (B, len(OP_CODES)) and freq_hz.shape == (F,)

    # Host precompute (tiny, fp64-exact): Laurent coefficients + omega powers
    coef = _host_coeffs(values)                              # (8, B, 13) f64
    om = 2.0 * np.pi * freq_hz.astype(np.float64)
    wt = om / OM0
    W = np.stack([wt ** (k - K0) for k in range(NK)]).astype(np.float32)

    if "nc" not in _COMPILED:
        _COMPILED["nc"] = _build_module()
    nc = _COMPILED["nc"]

    in_maps = []
    for core in range(NCORES):
        sl = slice(core * BPC, (core + 1) * BPC)
        cstack = np.ascontiguousarray(
            np.transpose(coef[:, sl, :], (2, 0, 1)).reshape(NK, CB)
        ).astype(np.float32)
        in_maps.append({"cst": cstack, "wst": W})

    res = run_bass_kernel_spmd(nc, in_maps, core_ids=list(range(NCORES)))
    LAST_RESULTS = res
    out = np.concatenate([res.results[c]["out"] for c in range(NCORES)], axis=1)
    return out.astype(np.float32, copy=False)


# revision 34
# speedup vs baseline: 1.1350x; 1.0216x over previous
"""Trainium2 kernel for nn_CascadedABCDCircuit: cascaded 2-port ABCD ladder.

Math: each stage multiplies the ABCD state by (I + s_i*G_i) where G_i is a
constant nilpotent 2x2 complex matrix and s_i = (omega*v_i)^{+-1}. Every
output component is therefore a Laurent polynomial in omega, degree -6..+6,
with batch-dependent coefficients. Host computes the 13 coefficients per
(component, batch) exactly in fp64 via the recurrence applied to polynomial
coefficient vectors (tiny (1024,13) complex ops). The device evaluates
out[c,b,f] = sum_m C[c,b,m] * W[m,f] as K=13 fp32r matmuls and streams the
256MB result to HBM — memory-bound, as this problem's regime demands.

Precision: plain fp32r (no hi/lo operand splits). fp32r rounds operands to
~12 mantissa bits; the measured end-to-end rel_l2 is ~1e-4, far inside the
2e-2 gate, and the single-term form cuts the input stream to 0.46MB/core.

Sharding: pure data-parallel over batch: 8 cores x 128 batches, every core
sees all 8192 freqs. The pipeline is tuned for the ~420 GB/s per-core DMA
write wall: production runs in 1024-col PSUM chunks (4-deep rotation)
copied by DVE/ACT alternately into 4096-col staging tiles (6 deep), each
shipped as ONE output DMA — 19 output DMAs total keeps the 8-deep HWDGE
completion-semaphore reuse window from ever stalling the two hardware
issue rings (sync/scalar). Groups go out f-block-major (every component's
[0:4096] block before any [4096:] block) so the first half of production
depends only on the first two wst loads — DGE completion semaphores have
a multi-us latency and mid-ramp waits on them starve the drain. The DRAM
pad keeps the output base in the HBM address phase where all 16 DMA
engines run at full per-engine rate (without it one engine runs ~20%
slow and becomes the long pole; the phase also depends on runtime
allocator state, so this is best-effort).
"""
import numpy as np
import sys

for _p in ("/opt/trn_rl_repo", "/root/.axon_site/_ro/trn_rl_repo"):
    if _p not in sys.path:
        sys.path.append(_p)

import concourse.bacc as bacc
import concourse.mybir as mybir
from concourse import tile
from concourse.bass_utils import run_bass_kernel_spmd

# Problem constants (hardcoded per contract)
B, F = 1024, 8192
OP_CODES = [3, 0, 1, 2, 3, 0, 1, 2, 3, 0, 1, 2]
Q_L, Q_C = 50.0, 100.0
NK, K0 = 13, 6               # omega powers -6..+6
NCORES = 8
BPC = B // NCORES            # 128 batches per core
NCOMP = 8                    # Ar, Ai, Br, Bi, Cr, Ci, Dr, Di
OM0 = 2.0 * np.pi * np.sqrt(1e9 * 10e9)   # omega normalizer (geometric mid)

MM_N = 512                   # max matmul moving cols (1 PSUM bank, fp32)
CHUNK = 1024                 # PSUM chunk cols (2 PSUM banks)
GRP = 4096                   # staging/DMA group cols
CB = NCOMP * BPC             # coefficient columns (1024)

LAST_RESULTS = None          # BassKernelResults of the most recent run
_COMPILED = {}


def _host_coeffs(values):
    """values (B,12) fp32 -> (NCOMP, B, NK) fp64 coeffs in powers of (om/OM0)."""
    v = values.astype(np.float64)
    nb = v.shape[0]
    A = np.zeros((nb, NK), np.complex128); A[:, K0] = 1.0
    Bm = np.zeros((nb, NK), np.complex128)
    Cm = np.zeros((nb, NK), np.complex128)
    Dm = np.zeros((nb, NK), np.complex128); Dm[:, K0] = 1.0

    def shift_mul(P, fac, dk):
        out = np.zeros_like(P)
        if dk == 1:
            out[:, 1:] = P[:, :-1]
        else:
            out[:, :-1] = P[:, 1:]
        return out * fac[:, None]

    for i, code in enumerate(OP_CODES):
        vi = v[:, i]
        if code == 0:      # series L
            fac = vi * OM0 * (1.0 / Q_L + 1j)
            Bm = Bm + shift_mul(A, fac, +1)
            Dm = Dm + shift_mul(Cm, fac, +1)
        elif code == 1:    # series C (reciprocal)
            c = (1.0 / Q_C - 1j) / (1.0 + 1.0 / Q_C**2)
            fac = c / (vi * OM0)
            Bm = Bm + shift_mul(A, fac, -1)
            Dm = Dm + shift_mul(Cm, fac, -1)
        elif code == 2:    # shunt L (reciprocal)
            c = (1.0 / Q_L - 1j) / (1.0 + 1.0 / Q_L**2)
            fac = c / (vi * OM0)
            A = A + shift_mul(Bm, fac, -1)
            Cm = Cm + shift_mul(Dm, fac, -1)
        else:              # shunt C
            fac = vi * OM0 * (1.0 / Q_C + 1j)
            A = A + shift_mul(Bm, fac, +1)
            Cm = Cm + shift_mul(Dm, fac, +1)
    return np.stack([A.real, A.imag, Bm.real, Bm.imag,
                     Cm.real, Cm.imag, Dm.real, Dm.imag])


def _group_schedule():
    """(c, pos, width) DMA groups, f-block-major: every component's
    [0:4096] block first, then the [4096:8192] blocks. The first half of
    production then depends only on the first two wst loads — no mid-ramp
    waits on load-completion semaphores. c0 leads with a small group so
    output DMA starts early; the last component tapers so the final queue
    drain is short."""
    groups = [(0, 0, 1024), (0, 1024, 1024), (0, 2048, 1024),
              (0, 3072, 1024)]
    for c in range(1, NCOMP):
        groups.append((c, 0, 4096))
    for c in range(NCOMP - 1):
        groups.append((c, 4096, 4096))
    groups += [(NCOMP - 1, 4096, 2048), (NCOMP - 1, 6144, 1024),
               (NCOMP - 1, 7168, 1024)]
    return groups


def _build_module():
    """SPMD module: cst[13,CB] + wst[13,F] -> out[NCOMP, BPC, F]."""
    nc = bacc.Bacc("TRN2", target_bir_lowering=False, debug=False,
                   enable_asserts=False, num_devices=NCORES)
    cst_d = nc.dram_tensor("cst", [NK, CB], mybir.dt.float32r,
                           kind="ExternalInput")
    wst_d = nc.dram_tensor("wst", [NK, F], mybir.dt.float32r,
                           kind="ExternalInput")
    # pad the DRAM layout so `out` starts at the same HBM offset class as
    # the fast baseline (inputs totalled 1.4375MB there); without this one
    # DMA engine's address slice lands ~24% slower and becomes the long pole
    pad_bytes = int(1.4375 * 1024 * 1024) - NK * CB * 4 - NK * F * 4
    nc.dram_tensor("pad", [pad_bytes // 4], mybir.dt.float32,
                   kind="Internal")
    out_d = nc.dram_tensor("out", [NCOMP, BPC, F], mybir.dt.float32,
                           kind="ExternalOutput")

    with tile.TileContext(nc) as tc:
        with (
            tc.tile_pool(name="const", bufs=1) as cpool,
            tc.tile_pool(name="stage", bufs=6) as spool,
            tc.tile_pool(name="ps", bufs=4, space="PSUM") as pspool,
        ):
            cst = cpool.tile([NK, CB], mybir.dt.float32r)
            wst = cpool.tile([NK, F], mybir.dt.float32r)
            # split load elements small (max_dma_last_dim) so every DGE
            # engine gets descriptors: a 13-row load otherwise leaves 3 of
            # 16 engines idle and its completion semaphore (all 16 must
            # report) lags the data by several us, starving the ramp
            nc.scalar.dma_start(wst[:, :1024], wst_d[:, :1024],
                                max_dma_last_dim=256)
            nc.sync.dma_start(cst[:, :BPC], cst_d[:, :BPC],
                              max_dma_last_dim=64)
            nc.scalar.dma_start(wst[:, 1024:4096], wst_d[:, 1024:4096],
                                max_dma_last_dim=512)
            nc.sync.dma_start(cst[:, BPC:], cst_d[:, BPC:],
                              max_dma_last_dim=256)
            nc.scalar.dma_start(wst[:, 4096:F], wst_d[:, 4096:F],
                                max_dma_last_dim=512)

            # Production runs in 1024-col PSUM chunks (4-deep rotation);
            # DMA is issued per 4096-col staging group: only 19 output DMAs,
            # so the 8-deep HWDGE completion-semaphore window (sem-reuse
            # guards on the in-order issue rings) spans ~16MB of drain.
            ncopy = 0
            for ng, (c, gpos, gw) in enumerate(_group_schedule()):
                ot = spool.tile([BPC, GRP], mybir.dt.float32)
                lhsT = cst[:, c * BPC:(c + 1) * BPC]
                nq = (gw + CHUNK - 1) // CHUNK
                for qi in range(nq):
                    q = qi * CHUNK
                    w = min(CHUNK, gw - q)
                    pos = gpos + q
                    acc = pspool.tile([BPC, CHUNK], mybir.dt.float32)
                    for j in range(0, w, MM_N):
                        mw = min(MM_N, w - j)
                        nc.tensor.matmul(acc[:, j:j + mw],
                                         lhsT, wst[:, pos + j:pos + j + mw])
                    # PSUM is only readable by DVE/ACT: alternate for copies
                    if ncopy % 2 == 0:
                        nc.vector.tensor_copy(ot[:, q:q + w], acc[:, :w])
                    else:
                        nc.scalar.copy(ot[:, q:q + w], acc[:, :w])
                    ncopy += 1
                # only sync and scalar own hardware DGE rings
                eng = nc.sync if ng % 2 == 0 else nc.scalar
                eng.dma_start(out_d[c, :, gpos:gpos + gw], ot[:, :gw])
    nc.compile()
    return nc


def kernel(values: np.ndarray, freq_hz: np.ndarray) -> np.ndarray:
    global LAST_RESULTS
    values = np.asarray(values, np.float32)
    freq_hz = np.asarray(freq_hz, np.float32)
    assert values.shape == (B, len(OP_CODES)) and freq_hz.shape == (F,)

    # Host precompute (tiny, fp64-exact): Laurent coefficients + omega powers
    coef = _host_coeffs(values)                              # (8, B, 13) f64
    om = 2.0 * np.pi * freq_hz.astype(np.float64)
    wt = om / OM0
    W = np.stack([wt ** (k - K0) for k in range(NK)]).astype(np.float32)

    if "nc" not in _COMPILED:
        _COMPILED["nc"] = _build_module()
    nc = _COMPILED["nc"]

    in_maps = []
    for core in range(NCORES):
        sl = slice(core * BPC, (core + 1) * BPC)
        cstack = np.ascontiguousarray(
            np.transpose(coef[:, sl, :], (2, 0, 1)).reshape(NK, CB)
        ).astype(np.float32)
        in_maps.append({"cst": cstack, "wst": W})

    res = run_bass_kernel_spmd(nc, in_maps, core_ids=list(range(NCORES)))
    LAST_RESULTS = res
    out = np.concatenate([res.results[c]["out"] for c in range(NCORES)], axis=1)
    return out.astype(np.float32, copy=False)
